# revision 32
# baseline (speedup 1.0000x reference)
"""BERT-CRF Viterbi decode kernel for Trainium2 (Bass/Tile), 8-core data parallel.

Full inputs in, full outputs out. Internally shards batch B=64 across 8 cores
(8 sequences each). Per core, scan rows r = b*16 + c (c = chunk of 32 steps):

  Stage A (u-grouped): one DMA per group of 4 steps (12KB contiguous per
  partition row), PE transposes h-chunks, batched fp32r matmul (W^T x sT)
  -> emissions^T in PSUM, bias folded into the ACT PSUM->SBUF copy,
  fix-transpose back to [rows, 4] landing in the emissions scan tile.
  Groups processed in order 0,7,1,6,2,5,3,4 so both scans below progress.

  Fused under stage A, two within-chunk scans in (max,+) algebra:
    fwd (DVE):  red_u[i,j]  = max_k(red_{u-1}[i,k] + e_{u-1}[k] + trans[k,j])
                (chunk-start tag i -> tag j at u, emissions e_0..e_{u-1})
    bwd (Pool): B_u[x,i]    = max_j(trans[i,j] + e_{u+1}[j] + B_{u+1}[x,j])
                (tag i at u -> chunk-end tag x, emissions e_{u+1}..e_{L-1})
  Each step is 2 ops (TT add + reduce_max) using per-group precomputed
  G_u[k,j] = e_u[k]+trans[k,j] (fwd) and G'_u[i,j] = trans[i,j]+e_u[j] (bwd).

  Tail: chunk-boundary chains (fwd scores on DVE, bwd suffix scores on Pool,
  concurrent), broadcast to rows, then tags for ALL timesteps at once:
    tag_u = first-argmax_j( max_i(sb[i]+red_u[i,j]) + e_u[j]
                            + max_x(B_u[x,j]+tb[x]) )
  via a handful of big [128, L*4] vector ops. No backpointer storage, no
  one-hot composition chains.
"""
import sys
for p in ("/opt/trn_rl_repo", "/root/.axon_site/_ro/trn_rl_repo"):
    if p not in sys.path:
        sys.path.append(p)

import numpy as np
import concourse.bass as bass
import concourse.tile as tile
from concourse import mybir
from concourse.bass_utils import run_bass_kernel_spmd

F32 = mybir.dt.float32
F32R = mybir.dt.float32r
I32 = mybir.dt.int32
AX = mybir.AxisListType
OP = mybir.AluOpType
AF = mybir.ActivationFunctionType

B, T, H, K = 64, 512, 768, 4
NCORES = 8
BC = B // NCORES          # 8 sequences per core
C, L = 16, 32             # chunks per sequence, steps per chunk
ROWS = BC * C             # 128 partition rows
HCH = H // 128            # 6 h-chunks
UG = 4                    # steps per u-group (batched matmul width 4*128=512)
NG = L // UG              # 8 groups
PORDER = [0, 7, 1, 6, 2, 5, 3, 4]
NEG = -1.0e9

_NC_CACHE = {}


def build_nc():
    nc = bass.Bass()
    sent = nc.declare_dram_parameter("sentences", [BC, T, H], F32, isOutput=False)
    Wd = nc.declare_dram_parameter("W", [K, H], F32, isOutput=False)
    identd = nc.declare_dram_parameter("identc", [128, 128], F32, isOutput=False)
    # rowconsts[128, 64]: wfirst | biascol | binit | end | ttr | tinit
    rcd = nc.declare_dram_parameter("rowconsts", [128, 64], F32, isOutput=False)
    tagsd = nc.declare_dram_parameter("tags", [BC, T], I32, isOutput=True)

    with tile.TileContext(nc) as tc:
        with tc.tile_pool(name="singles", bufs=1) as singles, \
             tc.tile_pool(name="sent_pool", bufs=3) as sent_pool, \
             tc.tile_pool(name="st_pool", bufs=2) as st_pool, \
             tc.tile_pool(name="dve_tmp", bufs=3) as dve_tmp, \
             tc.tile_pool(name="pl_tmp", bufs=3) as pl_tmp, \
             tc.tile_pool(name="ps_tr", bufs=3, space="PSUM") as ps_tr, \
             tc.tile_pool(name="ps_eT", bufs=2, space="PSUM") as ps_eT, \
             tc.tile_pool(name="ps_fix", bufs=2, space="PSUM") as ps_fix:

            # ---------- first step's sentences + constants (pipelined start) ----------
            # Group 0 is split into per-step DMAs so the first transpose can
            # start after ~400KB instead of ~1.6MB.
            pre_sg = sent_pool.tile([128, UG, H], F32, tag="sent")
            g0 = PORDER[0]
            for uu in range(UG):
                nc.sync.dma_start(pre_sg[:, uu, :], bass.AP(
                    tensor=sent[:].tensor, offset=(g0 * UG + uu) * H,
                    ap=[[T * H, BC], [L * H, C], [1, H]]))
                if uu == 0:
                    ident = singles.tile([128, 128], F32)
                    nc.sync.dma_start(ident, identd[:])
                    rc = singles.tile([128, 64], F32)
                    nc.sync.dma_start(rc, rcd[:])
            wfirst = rc[:, 0:4]
            biascol = rc[0:K, 4:5]
            binit_xi = rc[:, 8:24].rearrange("p (x i) -> p x i", x=4)
            end8 = rc[0:BC, 24:28]
            ttr = rc[:, 28:44]
            ttr_ij = ttr.rearrange("p (i j) -> p i j", i=4)
            tinit_ij = rc[:, 44:60].rearrange("p (i j) -> p i j", i=4)
            ident4 = rc[0:K, 60:64]

            # ---------- W^T in SBUF: wt[p = h within chunk, ch, k] ----------
            w_raw = singles.tile([K, H], F32)
            nc.sync.dma_start(w_raw, Wd[:])
            wt_sb = singles.tile([128, HCH, K], F32R)
            for ch in range(HCH):
                wt_ps = ps_fix.tile([128, UG * K], F32, tag="fq")
                nc.tensor.transpose(wt_ps[:, 0:K],
                                    w_raw[:, ch * 128:(ch + 1) * 128], ident4)
                nc.scalar.copy(wt_sb[:, ch, :], wt_ps[:, 0:K])

            # scan emissions tile (bias included), written by stage A
            emsc = singles.tile([128, L * K], F32)
            emv = emsc.rearrange("p (u j) -> p u j", u=L)

            # scan state/aux tiles
            G_all = singles.tile([128, L, 4, 4], F32)    # G_u[k,j]
            Gp_all = singles.tile([128, L, 4, 4], F32)   # G'_u[i,j]
            red_all = singles.tile([128, L, 4, 4], F32)  # red_u[i,j]
            B_all = singles.tile([128, L, 4, 4], F32)    # B_u[x,i]

            # ---------- helper emitters ----------
            def emsc_copy(g):
                nc.vector.tensor_copy(
                    emsc[:, g * UG * K:(g + 1) * UG * K], fq_tiles[g])

            def fwd_step(u):
                ftmp = dve_tmp.tile([128, 4, 4, 4], F32, tag="ftmp")
                # cand[i,j,k] = red_{u-1}[i,k] + G_{u-1}[k,j]
                nc.vector.tensor_tensor(
                    ftmp,
                    red_all[:, u - 1].unsqueeze(2).to_broadcast((128, 4, 4, 4)),
                    G_all[:, u - 1].transpose([0, 2, 1]).unsqueeze(1)
                        .to_broadcast((128, 4, 4, 4)),
                    OP.add)
                nc.vector.reduce_max(red_all[:, u], ftmp, axis=AX.X)

            def bwd_step(u):
                btmp = dve_tmp.tile([128, 4, 4, 4], F32, tag="btmp")
                # cand[x,i,j] = B_{u+1}[x,j] + G'_{u+1}[i,j]
                nc.vector.tensor_tensor(
                    btmp,
                    B_all[:, u + 1].unsqueeze(2).to_broadcast((128, 4, 4, 4)),
                    Gp_all[:, u + 1].unsqueeze(1).to_broadcast((128, 4, 4, 4)),
                    OP.add)
                nc.vector.reduce_max(B_all[:, u], btmp, axis=AX.X)

            def waves(lo, hi):
                for w in range(lo, hi):
                    fwd_step(w)
                    bwd_step(L - 1 - w)

            def g_ops(g):
                # G_u[k,j] = e_u[k]+trans[k,j]; G'_u[i,j] = trans[i,j]+e_u[j]
                nc.gpsimd.tensor_tensor(
                    G_all[:, g * UG:(g + 1) * UG],
                    emv[:, g * UG:(g + 1) * UG, :].unsqueeze(3)
                        .to_broadcast((128, UG, 4, 4)),
                    ttr_ij.unsqueeze(1).to_broadcast((128, UG, 4, 4)),
                    OP.add)
                nc.gpsimd.tensor_tensor(
                    Gp_all[:, g * UG:(g + 1) * UG],
                    ttr_ij.unsqueeze(1).to_broadcast((128, UG, 4, 4)),
                    emv[:, g * UG:(g + 1) * UG, :].unsqueeze(2)
                        .to_broadcast((128, UG, 4, 4)),
                    OP.add)

            # scan-block per position: emsc copies in arrival order, waves as
            # soon as their group pair (g, 7-g) has landed
            SCAN_BLOCK = {1: (1, 5), 3: (5, 9), 5: (9, 13), 7: (13, L)}

            # ---------- Stage A fused with scans, one interleaved stream ----------
            sA = nc.named_scope("stageA")
            sA.__enter__()
            nc.vector.tensor_copy(red_all[:, 0], tinit_ij)
            nc.vector.tensor_copy(B_all[:, L - 1], binit_xi)
            fq_tiles = {}
            for pos, g in enumerate(PORDER):
                if pos == 0:
                    sg = pre_sg
                else:
                    sg = sent_pool.tile([128, UG, H], F32, tag="sent")
                    nc.sync.dma_start(sg, bass.AP(
                        tensor=sent[:].tensor, offset=g * UG * H,
                        ap=[[T * H, BC], [L * H, C], [H, UG], [1, H]]))
                sT_sb = st_pool.tile([128, HCH, UG * 128], F32R, tag="sT")
                for ch in range(HCH):
                    trp = ps_tr.tile([128, UG * 128], F32, tag="trps")
                    for uu in range(UG):
                        nc.tensor.transpose(
                            trp[:, uu * 128:(uu + 1) * 128],
                            sg[:, uu, ch * 128:(ch + 1) * 128],
                            ident)
                    nc.scalar.copy(sT_sb[:, ch, :], trp)
                eT_ps = ps_eT.tile([4, UG * 128], F32, tag="eT")
                for ch in range(HCH):
                    nc.tensor.matmul(
                        eT_ps, wt_sb[:, ch, :], sT_sb[:, ch, :],
                        start=(ch == 0), stop=(ch == HCH - 1))
                # PSUM -> SBUF with bias add (b[k] per partition k) on DVE
                eT_sb = st_pool.tile([4, UG * 128], F32, tag="eTsb")
                nc.vector.tensor_tensor(
                    eT_sb, eT_ps, biascol.to_broadcast((K, UG * 128)), OP.add)
                fq = ps_fix.tile([128, UG * K], F32, tag="fq")
                for uu in range(UG):
                    nc.tensor.transpose(
                        fq[:, uu * K:(uu + 1) * K],
                        eT_sb[:, uu * 128:(uu + 1) * 128], ident4)
                fq_tiles[g] = fq
                emsc_copy(g)
                g_ops(g)
                if pos in SCAN_BLOCK:
                    waves(*SCAN_BLOCK[pos])
            sA.__exit__(None, None, None)

            # ---------- chunk matrices to by-b layout ----------
            sP2 = nc.named_scope("p2")
            sP2.__enter__()
            Ac = singles.tile([128, 16], F32)
            # Ac[i,j] = red_{L-1}[i,j] + e_{L-1}[j]
            nc.vector.tensor_tensor(
                Ac.rearrange("p (i j) -> p i j", i=4),
                red_all[:, L - 1],
                emv[:, L - 1, :].unsqueeze(1).to_broadcast((128, 4, 4)),
                OP.add)
            abyb = singles.tile([BC, C * 16], F32)
            nc.sync.dma_start(abyb, Ac)
            abv = abyb.rearrange("p (c i j) -> p c i j", c=C, i=4)

            # ---------- fwd boundary chain (DVE): sb_c per chunk ----------
            sbt = singles.tile([BC, 2 * C * 4], F32)
            sbv = sbt[:, 0:C * 4].rearrange("p (c j) -> p c j", c=C)
            tbv = sbt[:, C * 4:2 * C * 4].rearrange("p (c j) -> p c j", c=C)
            nc.vector.memset(sbt[:, 0:4], 0.0)
            for c in range(C - 1):
                p2tmp = dve_tmp.tile([BC, 4, 4], F32, tag="p2tmp")
                # tmp[j,i] = sb_c[i] + Ac_c[i,j]
                nc.vector.tensor_tensor(
                    p2tmp,
                    sbv[:, c, :].unsqueeze(1).to_broadcast((BC, 4, 4)),
                    abv[:, c].transpose([0, 2, 1]),
                    OP.add)
                nc.vector.reduce_max(sbv[:, c + 1, :], p2tmp, axis=AX.X)
            sP2.__exit__(None, None, None)

            # ---------- bwd boundary chain (DVE): tb_c per chunk ----------
            sTB = nc.named_scope("tb")
            sTB.__enter__()
            nc.vector.tensor_copy(tbv[:, C - 1, :], end8)
            for c in range(C - 2, -1, -1):
                ttmp = dve_tmp.tile([BC, 4, 4], F32, tag="ttmp")
                # tmp[x,j] = Ac_{c+1}[x,j] + tb_{c+1}[j]
                nc.vector.tensor_tensor(
                    ttmp,
                    abv[:, c + 1],
                    tbv[:, c + 1, :].unsqueeze(1).to_broadcast((BC, 4, 4)),
                    OP.add)
                nc.vector.reduce_max(tbv[:, c, :], ttmp, axis=AX.X)
            sbc = singles.tile([128, 4], F32)
            nc.sync.dma_start(sbc, sbt[:, 0:C * 4])
            tbc = singles.tile([128, 4], F32)
            nc.sync.dma_start(tbc, sbt[:, C * 4:2 * C * 4])
            sTB.__exit__(None, None, None)

            # ---------- combine: tags for all u at once ----------
            sCB = nc.named_scope("comb")
            sCB.__enter__()
            # Q_u[j] = max_x(B_u[x,j] + tb[x])   (TT on Pool, reduce on DVE)
            candQ = singles.tile([128, L, 4, 4], F32)
            nc.gpsimd.tensor_tensor(
                candQ,
                B_all.transpose([0, 1, 3, 2]),
                tbc.unsqueeze(1).unsqueeze(1).to_broadcast((128, L, 4, 4)),
                OP.add)
            Q = singles.tile([128, L, 4], F32)
            nc.vector.reduce_max(Q, candQ, axis=AX.X)

            # P_u[j] = max_i(sb[i] + red_u[i,j]) + e_u[j]   (DVE)
            candP = singles.tile([128, L, 4, 4], F32)
            nc.vector.tensor_tensor(
                candP,
                red_all.transpose([0, 1, 3, 2]),
                sbc.unsqueeze(1).unsqueeze(1).to_broadcast((128, L, 4, 4)),
                OP.add)
            P = singles.tile([128, L, 4], F32)
            nc.vector.reduce_max(P, candP, axis=AX.X)
            R = singles.tile([128, L, 4], F32)
            nc.vector.tensor_tensor(R, P, emv, OP.add)
            nc.vector.tensor_tensor(R, R, Q, OP.add)
            M = singles.tile([128, L], F32)
            nc.vector.reduce_max(M, R, axis=AX.X)
            eq = singles.tile([128, L, 4], F32)
            nc.vector.tensor_tensor(
                eq, R, M.unsqueeze(2).to_broadcast((128, L, 4)), OP.is_equal)
            nc.vector.tensor_tensor(
                eq, eq, wfirst.unsqueeze(1).to_broadcast((128, L, 4)), OP.mult)
            Wm = singles.tile([128, L], F32)
            nc.vector.reduce_max(Wm, eq, axis=AX.X)
            tagf = singles.tile([128, L], F32)
            nc.vector.tensor_scalar(tagf, Wm, -1.0, 4.0, OP.mult, OP.add)
            tagi = singles.tile([128, L], I32)
            nc.vector.tensor_copy(tagi, tagf)
            nc.sync.dma_start(tagsd[:].rearrange("b (c t) -> b c t", c=C), tagi)
            sCB.__exit__(None, None, None)

    return nc


def _split_multi_waits(nc):
    """Walrus (bass2jax path) allows very few embedded sync waits per
    instruction (PE matmul: exactly 1). Hoist multi-waits onto standalone
    single-wait InstDrain instructions on the same engine, preserving order."""
    for f in nc.m.functions:
        for blk in f.blocks:
            insts = blk.instructions
            i = 0
            while i < len(insts):
                ins = insts[i]
                si = ins.sync_info
                w = list(si.on_wait) if (si is not None and si.on_wait) else []
                if len(w) >= 2:
                    for k, wait in enumerate(w):
                        d = mybir.InstEventSemaphore(
                            name=nc.get_next_instruction_name(), ins=[], outs=[])
                        d.engine = ins.engine
                        d.sync_info = mybir.SyncInfo(on_wait=[wait], on_update=[])
                        insts.insert(i + k, d)
                    i += len(w)
                    ins.sync_info = mybir.SyncInfo(
                        on_wait=[], on_update=list(si.on_update or []))
                i += 1


def _get_nc():
    if "nc" not in _NC_CACHE:
        nc = build_nc()
        _split_multi_waits(nc)   # HW path only; CoreSim rejects raw drains
        _NC_CACHE["nc"] = nc
    return _NC_CACHE["nc"]


def make_in_maps(inputs):
    sent = np.ascontiguousarray(np.asarray(inputs["sentences"], dtype=np.float32))
    W = np.ascontiguousarray(np.asarray(inputs["W"], dtype=np.float32))
    bb = np.ascontiguousarray(np.asarray(inputs["b"], dtype=np.float32))
    st = np.ascontiguousarray(np.asarray(inputs["start_transitions"], dtype=np.float32))
    en = np.ascontiguousarray(np.asarray(inputs["end_transitions"], dtype=np.float32))
    tr = np.ascontiguousarray(np.asarray(inputs["transitions"], dtype=np.float32))
    tinit = np.tile(tr.ravel(), (128, 1)).astype(np.float32)
    tinit[0::C, :] = np.tile(st, 4)[None, :]
    binit = np.full((4, 4), NEG, dtype=np.float32)
    np.fill_diagonal(binit, 0.0)
    rc = np.zeros((128, 64), dtype=np.float32)
    rc[:, 0:4] = [4.0, 3.0, 2.0, 1.0]
    rc[0:K, 4] = bb
    rc[:, 8:24] = binit.ravel()[None, :]
    rc[:, 24:28] = en[None, :]
    rc[:, 28:44] = tr.ravel()[None, :]
    rc[:, 44:60] = tinit
    rc[0:K, 60:64] = np.eye(K, dtype=np.float32)
    identc = np.eye(128, dtype=np.float32)
    return [{
        "sentences": sent[c * BC:(c + 1) * BC],
        "W": W, "identc": identc, "rowconsts": rc,
    } for c in range(NCORES)]


def kernel(**inputs):
    nc = _get_nc()
    in_maps = make_in_maps(inputs)
    res = run_bass_kernel_spmd(nc, in_maps, core_ids=list(range(NCORES)))
    tags = np.concatenate([res.results[c]["tags"] for c in range(NCORES)], axis=0)
    return tags.astype(np.int32)


if __name__ == "__main__":
    import reference
    inputs = {k: np.asarray(v) for k, v in reference.setup_inputs().items()}
    out = kernel(**inputs)
    print(out.shape, out.dtype, out[:2, :16])
